# revision 1
# baseline (speedup 1.0000x reference)
"""Chamfer distance kernel for Trainium2 (8 NeuronCores, SPMD).

Problem: B=4 batches, N=M=8192 points, D=3. Per batch:
    d2[n,m] = ||a_n - b_m||^2  (clamped at 0)
    out[b]  = mean_n(min_m d2) + mean_m(min_n d2)

Sharding: core c handles batch c//2, rows [h*4096,(h+1)*4096) of pc1 (h=c%2).
Each core computes, for its 4096x8192 block of the distance matrix:
  - rowmins: per-row min over all 8192 columns         -> [128, 32] fp32
  - colacc : per-column min over its 4096 rows (as a
             128-partition-wise partial min)            -> [128, 8192] fp16
Host combines: full col-min = min over partitions and over the 2 cores of a
batch; means are tiny host-side reductions.

On-core pipeline per 128-row tile (32 tiles, processed in pairs):
  PE    : 16 matmuls K=11 fp16 hi/lo-split -> psum = -2 a.b + ||b||^2 (fp32).
          The hi/lo split ([-2a_hi, -2a_lo, -2a_hi, 1, 1] x
          [b_hi, b_hi, b_lo, b2_hi, b2_lo]) reproduces the fp32 product to
          ~1e-5 absolute while running at full fp16 PE rate (fp32 matmul is
          4x slower; fp32r has accuracy caveats).
  ScalarE: relu(psum + ||a||^2 per-partition bias) -> fp16 SBUF. This is the
          only PSUM->SBUF escape path that does not burn VectorE cycles, and
          the fused relu reproduces the reference's maximum(d2, 0).
  VectorE: col-min fold (tensor_tensor min, 2x_1p on fp16) + row-min binary
          tree (tensor_tensor min levels + one small tensor_reduce), with
          tree levels shared across the tile pair via [128, 2, w] APs.

Both VectorE (2 passes/element at 2 elem/cycle/lane) and ScalarE (1 pass at
1 elem/cycle/lane) run at their architectural floors. Cost-model timeline:
282 us steady + 22 us fixed = 304 us/core; HW reps-slope measurements of the
main loop across sessions: 190/208/234/252/266 us (median ~235 us, axon-proxy
noise), so expect ~255 us total on quiet hardware, <= 305 us worst case.
Accuracy: 4.7e-05 max relative error vs the fp32 reference.
"""

import numpy as np

B, N, M, D = 4, 8192, 8192, 3
NCORES = 8
NH = N // 2          # rows per core
NT = NH // 128       # 32 n-tiles of 128 rows
K = 11               # split-matmul contraction size

_CACHE = {}


def _build(reps=1, no_tree=False, no_fold=False, no_escape=False,
           paired=False, tiny_out=False, d2_bufs=2, tree_bufs=2,
           alloc_mode="stack"):
    """Build + compile the SPMD NEFF once per process.

    reps>1 repeats the main loop (identical results) — used only for
    slope-based execution timing; the product path uses reps=1.
    no_tree/no_fold/no_escape build ablation variants for engine-bottleneck
    analysis (wrong results, timing only).
    """
    import concourse.bacc as bacc
    import concourse.tile as tile
    import concourse.mybir as mybir

    nc = bacc.Bacc("TRN2", target_bir_lowering=False, debug=False,
                   num_devices=NCORES)
    f16, f32 = mybir.dt.float16, mybir.dt.float32

    w_d = nc.dram_tensor("w", [K, NH], f16, kind="ExternalInput")
    bh_d = nc.dram_tensor("bh", [K, M], f16, kind="ExternalInput")
    a2_d = nc.dram_tensor("a2", [128, NT], f32, kind="ExternalInput")
    colacc_shape = [128, 32] if tiny_out else [128, M]
    colacc_d = nc.dram_tensor("colacc", colacc_shape, f16,
                              kind="ExternalOutput")
    rowmins_d = nc.dram_tensor("rowmins", [128, NT], f32, kind="ExternalOutput")

    tmin = mybir.AluOpType.min

    with tile.TileContext(nc, pool_alloc_mode=alloc_mode) as tc:
        with (
            tc.tile_pool(name="consts", bufs=1) as consts,
            tc.tile_pool(name="psum", bufs=2, space="PSUM") as psum_pool,
            tc.tile_pool(name="d2", bufs=d2_bufs) as d2_pool,
            tc.tile_pool(name="tree", bufs=tree_bufs) as tree_pool,
        ):
            w_sb = consts.tile([K, NH], f16)
            nc.sync.dma_start(out=w_sb, in_=w_d.ap())
            bh_sb = consts.tile([K, M], f16)
            nc.sync.dma_start(out=bh_sb, in_=bh_d.ap())
            a2_sb = consts.tile([128, NT], f32)
            nc.sync.dma_start(out=a2_sb, in_=a2_d.ap())

            colacc = consts.tile([128, M], f16)
            rowmins = consts.tile([128, NT], f32)

            if paired:
                assert not (no_tree or no_fold or no_escape)
                _build_paired(nc, tc, mybir, reps, d2_pool, psum_pool,
                              tree_pool, w_sb, bh_sb, a2_sb, colacc, rowmins)
            else:
                _build_plain(nc, tc, mybir, reps, no_tree, no_fold, no_escape,
                             d2_pool, psum_pool, tree_pool, w_sb, bh_sb, a2_sb,
                             colacc, rowmins)

            if tiny_out:
                nc.sync.dma_start(out=colacc_d.ap(), in_=colacc[:, :32])
            else:
                nc.sync.dma_start(out=colacc_d.ap(), in_=colacc)
            nc.sync.dma_start(out=rowmins_d.ap(), in_=rowmins)

    nc.compile()
    return nc


def _build_plain(nc, tc, mybir, reps, no_tree, no_fold, no_escape,
                 d2_pool, psum_pool, tree_pool, w_sb, bh_sb, a2_sb,
                 colacc, rowmins):
    f16, f32 = mybir.dt.float16, mybir.dt.float32
    tmin = mybir.AluOpType.min
    if True:
        if True:
            for i in [t for _ in range(reps) for t in range(NT)]:
                d2row = d2_pool.tile([128, M], f16)
                for q in range(4):
                    ps = psum_pool.tile([128, 2048], f32)
                    for jj in range(4):
                        j = q * 4 + jj
                        nc.tensor.matmul(
                            ps[:, jj * 512:(jj + 1) * 512],
                            w_sb[:, i * 128:(i + 1) * 128],
                            bh_sb[:, j * 512:(j + 1) * 512],
                            start=True, stop=True,
                        )
                    if not no_escape:
                        nc.scalar.activation(
                            out=d2row[:, q * 2048:(q + 1) * 2048],
                            in_=ps,
                            func=mybir.ActivationFunctionType.Relu,
                            bias=a2_sb[:, i:i + 1],
                            scale=1.0,
                        )
                if no_escape:
                    nc.gpsimd.memset(d2row, 1.0)
                # direction-2: fold this row-block into the column-min accum
                if i == 0:
                    nc.vector.tensor_copy(out=colacc, in_=d2row)
                elif not no_fold:
                    nc.vector.tensor_tensor(out=colacc, in0=colacc,
                                            in1=d2row, op=tmin)
                # direction-1: row-min tree over the 8192 columns
                if no_tree and i == 0:
                    nc.gpsimd.memset(rowmins, 0.0)
                if not no_tree:
                    tr = tree_pool.tile([128, 4096], f16)
                    nc.vector.tensor_tensor(out=tr, in0=d2row[:, :4096],
                                            in1=d2row[:, 4096:], op=tmin)
                    nc.vector.tensor_tensor(out=tr[:, :2048], in0=tr[:, :2048],
                                            in1=tr[:, 2048:4096], op=tmin)
                    nc.vector.tensor_tensor(out=tr[:, :1024], in0=tr[:, :1024],
                                            in1=tr[:, 1024:2048], op=tmin)
                    nc.vector.tensor_tensor(out=tr[:, :512], in0=tr[:, :512],
                                            in1=tr[:, 512:1024], op=tmin)
                    nc.vector.tensor_tensor(out=tr[:, :256], in0=tr[:, :256],
                                            in1=tr[:, 256:512], op=tmin)
                    nc.vector.tensor_tensor(out=tr[:, :128], in0=tr[:, :128],
                                            in1=tr[:, 128:256], op=tmin)
                    nc.vector.tensor_tensor(out=tr[:, :64], in0=tr[:, :64],
                                            in1=tr[:, 64:128], op=tmin)
                    nc.vector.tensor_reduce(out=rowmins[:, i:i + 1],
                                            in_=tr[:, :64],
                                            axis=mybir.AxisListType.X, op=tmin)


def _build_paired(nc, tc, mybir, reps, d2_pool, psum_pool, tree_pool,
                  w_sb, bh_sb, a2_sb, colacc, rowmins):
    """2 n-tiles per DVE op-group: tree levels run on [128, 2, w] APs."""
    f16, f32 = mybir.dt.float16, mybir.dt.float32
    tmin = mybir.AluOpType.min
    for ii in [t for _ in range(reps) for t in range(NT // 2)]:
        d2p = d2_pool.tile([128, 2, M], f16, tag="d2p")
        for half in range(2):
            i = 2 * ii + half
            for q in range(4):
                ps = psum_pool.tile([128, 2048], f32, tag="ps")
                for jj in range(4):
                    j = q * 4 + jj
                    nc.tensor.matmul(
                        ps[:, jj * 512:(jj + 1) * 512],
                        w_sb[:, i * 128:(i + 1) * 128],
                        bh_sb[:, j * 512:(j + 1) * 512],
                        start=True, stop=True,
                    )
                nc.scalar.activation(
                    out=d2p[:, half, q * 2048:(q + 1) * 2048],
                    in_=ps,
                    func=mybir.ActivationFunctionType.Relu,
                    bias=a2_sb[:, i:i + 1],
                    scale=1.0,
                )
            # fold each half into colacc as soon as it is escaped
            if i == 0:
                nc.vector.tensor_copy(out=colacc, in_=d2p[:, 0, :])
            else:
                nc.vector.tensor_tensor(out=colacc, in0=colacc,
                                        in1=d2p[:, half, :], op=tmin)
        # paired row-min tree over both n-tiles at once
        tr = tree_pool.tile([128, 2, 4096], f16, tag="trp")
        nc.vector.tensor_tensor(out=tr, in0=d2p[:, :, :4096],
                                in1=d2p[:, :, 4096:], op=tmin)
        nc.vector.tensor_tensor(out=tr[:, :, :2048], in0=tr[:, :, :2048],
                                in1=tr[:, :, 2048:4096], op=tmin)
        nc.vector.tensor_tensor(out=tr[:, :, :1024], in0=tr[:, :, :1024],
                                in1=tr[:, :, 1024:2048], op=tmin)
        nc.vector.tensor_tensor(out=tr[:, :, :512], in0=tr[:, :, :512],
                                in1=tr[:, :, 512:1024], op=tmin)
        nc.vector.tensor_tensor(out=tr[:, :, :256], in0=tr[:, :, :256],
                                in1=tr[:, :, 256:512], op=tmin)
        # keep halving at 2x down to 64 before the 1x-only tensor_reduce
        nc.vector.tensor_tensor(out=tr[:, :, :128], in0=tr[:, :, :128],
                                in1=tr[:, :, 128:256], op=tmin)
        nc.vector.tensor_tensor(out=tr[:, :, :64], in0=tr[:, :, :64],
                                in1=tr[:, :, 64:128], op=tmin)
        nc.vector.tensor_reduce(out=rowmins[:, 2 * ii:2 * ii + 2],
                                in_=tr[:, :, :64],
                                axis=mybir.AxisListType.X, op=tmin)


def _prep_inputs(pc1, pc2):
    """Host-side: build per-core fp16 hi/lo split operands (tiny arrays)."""
    in_maps = []
    for c in range(NCORES):
        b, h = divmod(c, 2)
        a = np.asarray(pc1[b][h * NH:(h + 1) * NH], dtype=np.float32)  # [NH,3]
        bb = np.asarray(pc2[b], dtype=np.float32)                      # [M,3]

        ah = a.astype(np.float16)
        al = (a - ah.astype(np.float32)).astype(np.float16)
        w = np.empty((K, NH), dtype=np.float16)
        w[0:3] = (ah.T * np.float16(-2))
        w[3:6] = (al.T * np.float16(-2))
        w[6:9] = (ah.T * np.float16(-2))
        w[9] = np.float16(1.0)
        w[10] = np.float16(1.0)

        bhh = bb.astype(np.float16)
        bl = (bb - bhh.astype(np.float32)).astype(np.float16)
        b2 = np.square(bb.astype(np.float64)).sum(-1)                  # [M]
        b2h = b2.astype(np.float16)
        b2l = (b2 - b2h.astype(np.float64)).astype(np.float16)
        bh = np.empty((K, M), dtype=np.float16)
        bh[0:3] = bhh.T
        bh[3:6] = bhh.T
        bh[6:9] = bl.T
        bh[9] = b2h
        bh[10] = b2l

        a2 = np.square(a.astype(np.float64)).sum(-1).astype(np.float32)
        a2 = np.ascontiguousarray(a2.reshape(NT, 128).T)               # [128,NT]

        in_maps.append({"w": w, "bh": bh, "a2": a2})
    return in_maps


def _run(in_maps, trace=False):
    from concourse.bass_utils import run_bass_kernel_spmd
    if "nc" not in _CACHE:
        _CACHE["nc"] = _build(paired=True)
    return run_bass_kernel_spmd(_CACHE["nc"], in_maps,
                                core_ids=list(range(NCORES)), trace=trace)


def kernel(pc1, pc2, _trace=False):
    pc1 = np.asarray(pc1, dtype=np.float32)
    pc2 = np.asarray(pc2, dtype=np.float32)
    res = _run(_prep_inputs(pc1, pc2), trace=_trace)

    out = np.empty((B,), dtype=np.float32)
    for b in range(B):
        r0, r1 = res.results[2 * b], res.results[2 * b + 1]
        colmin = np.minimum(
            r0["colacc"].astype(np.float32).min(axis=0),
            r1["colacc"].astype(np.float32).min(axis=0),
        )                                                              # [M]
        term2 = colmin.mean(dtype=np.float64)
        rowmins = np.concatenate([r0["rowmins"].ravel(),
                                  r1["rowmins"].ravel()])
        term1 = rowmins.mean(dtype=np.float64)
        out[b] = np.float32(term1 + term2)
    kernel._last_results = res
    return out

